# revision 1
# baseline (speedup 1.0000x reference)
"""Trainium2 Bass kernel for BasicRecurrentEntityEncoder.

Math (per batch b, entity k, step t):
  enc[b,t,:]  = sum_l mask[b,t,l] * emb[prgrph[b,t,l]] * posmask[l,:]
  g           = sigmoid((h+keys)·s) * sent_mask          (mask folded into gate)
  h_tilda     = sigmoid(h@U + keys@V + s@W)
  h           = normalize(h + g*h_tilda)                  (exact when g=0: h is 0 or unit)

Sharding: data-parallel over batch, 8 paragraphs per core.

Per-core on-chip layouts (BL=8 local paragraphs, K=64, D=128 -> 512 state cols):
  feature-major: col c = b*64 + k, tiles [D=128, 512]     (for PE matmuls)
  layout-B:      chunk j = c>>7, partition p = c&127      (for per-(b,k) scalar ops)
                 so b = 2j + (p>>6), k = p&63

Scan step engines: PE does U/V/W matmuls, gate row-dots, transposes;
ACT does sigmoids + psum->sbuf copy; DVE does gate select, the gated update
(scalar_tensor_tensor), squared-norm (tensor_tensor_reduce), and an
rsqrt via int32-domain magic seed + 2 Newton iterations (ACT Rsqrt is banned
and lives in a different activation-table set than Sigmoid anyway).
"""
import numpy as np

import concourse.bass as bass
import concourse.bacc as bacc
import concourse.tile as tile
from concourse import mybir
from concourse.bass_utils import run_bass_kernel_spmd

F32 = mybir.dt.float32
I32 = mybir.dt.int32
AF = mybir.ActivationFunctionType
ALU = mybir.AluOpType

B, T, L, D, K, V = 64, 128, 32, 128, 64, 50000
NCORES = 8
BL = B // NCORES              # 8 paragraphs per core
COLS = BL * K                 # 512 state columns per core
NJ = COLS // 128              # 4 layout-B chunks
WORDS = BL * T * L            # 32768 gathered words per core
CHUNKS = WORDS // 128         # 256
G = 8                         # chunks per gather instruction
NGI = CHUNKS // G             # 32 gather instructions
MAGIC = 0x5F3759DF

_cache = {}

# debug knobs: restrict which phases are built
DBG_PHASE1 = True
DBG_SCAN_T = T
DBG_LVL = 9  # 1: mm+sigmoid; 2: +gate mms+transposes; 3: +gate small ops;
             # 4: +STT hn; 5: +TTR ss; 6: +rsqrt; 7: +apply; 9: full


def _build_nc():
    nc = bacc.Bacc(None, target_bir_lowering=False)

    emb_t = nc.dram_tensor("emb", [V, D], F32, kind="ExternalInput")
    gidx_t = nc.dram_tensor("gidx", [NGI, 128, G], I32, kind="ExternalInput")
    mo_t = nc.dram_tensor("maskones", [NGI, 128, G, 4], F32, kind="ExternalInput")
    posrep_t = nc.dram_tensor("posrep", [128, 128], F32, kind="ExternalInput")
    keysT_t = nc.dram_tensor("keysT", [128, COLS], F32, kind="ExternalInput")
    U_t = nc.dram_tensor("Uw", [D, D], F32, kind="ExternalInput")
    V_t = nc.dram_tensor("Vw", [D, D], F32, kind="ExternalInput")
    W_t = nc.dram_tensor("Ww", [D, D], F32, kind="ExternalInput")
    mscal_t = nc.dram_tensor("maskscal", [128, 4 * T], F32, kind="ExternalInput")
    oh_t = nc.dram_tensor("onehot32", [128, 32], F32, kind="ExternalInput")
    id_t = nc.dram_tensor("ident", [128, 128], F32, kind="ExternalInput")
    out_t = nc.dram_tensor("h_out", [BL, K, D], F32, kind="ExternalOutput")

    with tile.TileContext(nc) as tc:
        with tc.tile_pool(name="persist", bufs=1) as pp:
            posrep = pp.tile([128, 128], F32)
            keysT = pp.tile([128, COLS], F32)
            Uw = pp.tile([D, D], F32)
            Vw = pp.tile([D, D], F32)
            Ww = pp.tile([D, D], F32)
            mscal = pp.tile([128, 4 * T], F32)      # [p, 4t+j] sentence mask
            oh32 = pp.tile([128, 32], F32)
            ident = pp.tile([128, 128], F32)
            encT = pp.tile([128, T * BL], F32)      # [d, t*8+b]
            ksst = pp.tile([128, 4 * T], F32)       # [p, 4t+j]
            nc.sync.dma_start(out=posrep, in_=posrep_t[:, :])
            nc.sync.dma_start(out=keysT, in_=keysT_t[:, :])
            nc.sync.dma_start(out=Uw, in_=U_t[:, :])
            nc.sync.dma_start(out=Vw, in_=V_t[:, :])
            nc.sync.dma_start(out=Ww, in_=W_t[:, :])
            nc.sync.dma_start(out=mscal, in_=mscal_t[:, :])
            nc.sync.dma_start(out=oh32, in_=oh_t[:, :])
            nc.sync.dma_start(out=ident, in_=id_t[:, :])

            # ---------------- Phase 1: gather + sentence encoder ----------
            with tc.tile_pool(name="p1sb", bufs=3) as p1, \
                 tc.tile_pool(name="p1w", bufs=3) as p1w, \
                 tc.tile_pool(name="p1ps", bufs=2, space="PSUM") as p1ps:
                penc = None
                for n in range(NGI if DBG_PHASE1 else 0):
                    idx = p1.tile([128, G], I32, tag="idx")
                    nc.sync.dma_start(out=idx, in_=gidx_t[n, :, :])
                    mo = p1.tile([128, G, 4], F32, tag="mo")
                    nc.sync.dma_start(out=mo, in_=mo_t[n, :, :, :])
                    embg = p1.tile([128, G, 128], F32, tag="embg")
                    for g in range(G):
                        nc.gpsimd.indirect_dma_start(
                            out=embg[:, g, :], out_offset=None, in_=emb_t[:, :],
                            in_offset=bass.IndirectOffsetOnAxis(
                                ap=idx[:, g:g + 1], axis=0))
                    for g in range(G):
                        ch = n * G + g
                        if ch % 32 == 0:
                            penc = p1ps.tile([128, 128], F32, tag="penc")
                        wt = p1w.tile([128, 128], F32, tag="wt")
                        nc.vector.tensor_tensor(
                            out=wt, in0=embg[:, g, :], in1=posrep, op=ALU.mult)
                        nc.tensor.matmul(
                            out=penc[:, (ch % 32) * 4:(ch % 32) * 4 + 4],
                            lhsT=wt, rhs=mo[:, g, :], start=True, stop=True)
                        if ch % 32 == 31:
                            nc.scalar.copy(
                                out=encT[:, (ch // 32) * 128:(ch // 32) * 128 + 128],
                                in_=penc)

            # ---------------- Phase 1.5: ks table -------------------------
            # ks[b,k,t] = sum_d keys[b,k,d]*enc[b,t,d], stored [p, 4t+j]
            with tc.tile_pool(name="ksps", bufs=2, space="PSUM") as ksps:
                for b in range(BL if DBG_PHASE1 else 0):
                    psk = ksps.tile([64, 128], F32, tag="psk")
                    encb = bass.AP(tensor=encT.tensor, offset=encT.offset + b,
                                   ap=[encT.ap[0], [BL, T]])
                    nc.tensor.matmul(out=psk, lhsT=keysT[:, b * 64:(b + 1) * 64],
                                     rhs=encb, start=True, stop=True)
                    nc.vector.tensor_copy(
                        out=ksst[(b & 1) * 64:(b & 1) * 64 + 64, (b >> 1)::4],
                        in_=psk)

            # ---------------- Phase 2: the scan ---------------------------
            with tc.tile_pool(name="st", bufs=2) as stp, \
                 tc.tile_pool(name="sm", bufs=3) as smp, \
                 tc.tile_pool(name="scr", bufs=2) as scrp, \
                 tc.tile_pool(name="psA", bufs=2, space="PSUM") as psA, \
                 tc.tile_pool(name="psB", bufs=2, space="PSUM") as psB, \
                 tc.tile_pool(name="psG", bufs=2, space="PSUM") as psG, \
                 tc.tile_pool(name="psH", bufs=2, space="PSUM") as psH:
                hT = stp.tile([128, COLS], F32, tag="hT")
                hB = stp.tile([128, COLS], F32, tag="hB")
                nc.vector.memset(hT, 0.0)
                nc.vector.memset(hB, 0.0)
                if not DBG_PHASE1:
                    nc.vector.memset(encT, 0.0)
                    nc.vector.memset(ksst, 0.0)

                for t in range(DBG_SCAN_T):
                    s_sl = encT[:, 8 * t:8 * t + 8]
                    # pre-activation: U.T@hT + V.T@keysT + W.T@bcast(s)
                    pA = psA.tile([128, COLS], F32, tag="pA")
                    nc.tensor.matmul(out=pA, lhsT=Uw, rhs=hT,
                                     start=True, stop=False)
                    nc.tensor.matmul(out=pA, lhsT=Vw, rhs=keysT,
                                     start=False, stop=False)
                    s_bc = bass.AP(tensor=encT.tensor,
                                   offset=encT.offset + 8 * t,
                                   ap=[encT.ap[0], [1, BL], [0, K]])
                    nc.tensor.matmul(out=pA, lhsT=Ww, rhs=s_bc,
                                     start=False, stop=True)
                    htT = scrp.tile([128, COLS], F32, tag="htT")
                    nc.scalar.activation(out=htT, in_=pA, func=AF.Sigmoid)
                    if DBG_LVL < 2:
                        continue

                    # gate row-dots: pG[:, 8j+b'] = sum_d hT[d, 128j+p]*s[d,b']
                    pG = psG.tile([128, 32], F32, tag="pG")
                    for j in range(NJ):
                        nc.tensor.matmul(out=pG[:, 8 * j:8 * j + 8],
                                         lhsT=hT[:, 128 * j:128 * (j + 1)],
                                         rhs=s_sl, start=True, stop=True)
                    # transpose h_tilda into layout-B
                    pB = psB.tile([128, COLS], F32, tag="pB")
                    for j in range(NJ):
                        nc.tensor.transpose(out=pB[:, 128 * j:128 * (j + 1)],
                                            in_=htT[:, 128 * j:128 * (j + 1)],
                                            identity=ident)
                    if DBG_LVL < 3:
                        gsc = scrp.tile([128, COLS], F32, tag="gsc")
                        nc.vector.tensor_copy(out=gsc, in_=pB)
                        continue

                    gsel = smp.tile([128, 32], F32, tag="gsel")
                    nc.vector.tensor_tensor(out=gsel, in0=pG, in1=oh32,
                                            op=ALU.mult)
                    graw = smp.tile([128, 4], F32, tag="graw")
                    nc.vector.tensor_reduce(
                        out=graw, in_=gsel.rearrange("p (a b) -> p a b", b=8),
                        axis=mybir.AxisListType.X, op=ALU.add)
                    gks = smp.tile([128, 4], F32, tag="gks")
                    nc.vector.tensor_tensor(out=gks, in0=graw,
                                            in1=ksst[:, 4 * t:4 * t + 4],
                                            op=ALU.add)
                    gs = smp.tile([128, 4], F32, tag="gs")
                    nc.scalar.activation(out=gs, in_=gks, func=AF.Sigmoid)
                    gm = smp.tile([128, 4], F32, tag="gm")
                    nc.vector.tensor_tensor(out=gm, in0=gs,
                                            in1=mscal[:, 4 * t:4 * t + 4],
                                            op=ALU.mult)
                    if DBG_LVL < 4:
                        continue

                    # hn = h + g*h_tilda  (layout B)
                    hnB = scrp.tile([128, COLS], F32, tag="hnB")
                    for j in range(NJ):
                        nc.vector.scalar_tensor_tensor(
                            out=hnB[:, 128 * j:128 * (j + 1)],
                            in0=pB[:, 128 * j:128 * (j + 1)],
                            scalar=gm[:, j:j + 1],
                            in1=hB[:, 128 * j:128 * (j + 1)],
                            op0=ALU.mult, op1=ALU.add)
                    if DBG_LVL < 5:
                        continue
                    # ss = sum_d hn^2  (tensor_tensor_reduce miscomputes on HW;
                    # use square + free-dim reduce instead)
                    ss = smp.tile([128, 4], F32, tag="ss")
                    sq = scrp.tile([128, COLS], F32, tag="sq")
                    nc.vector.tensor_tensor(out=sq, in0=hnB, in1=hnB,
                                            op=ALU.mult)
                    nc.vector.tensor_reduce(
                        out=ss, in_=sq.rearrange("p (a b) -> p a b", b=128),
                        axis=mybir.AxisListType.X, op=ALU.add)
                    if DBG_LVL < 6:
                        continue
                    ssc = smp.tile([128, 4], F32, tag="ssc")
                    nc.vector.tensor_scalar(out=ssc, in0=ss, scalar1=1e-12,
                                            scalar2=None, op0=ALU.max)
                    # inv = rsqrt(ssc): magic seed (int32 value domain) + 2 NR
                    seed = smp.tile([128, 4], I32, tag="seed")
                    nc.vector.tensor_scalar(out=seed, in0=ssc.bitcast(I32),
                                            scalar1=-0.5, scalar2=float(MAGIC),
                                            op0=ALU.mult, op1=ALU.add)
                    y0 = seed.bitcast(F32)
                    t1 = smp.tile([128, 4], F32, tag="t1")
                    t2 = smp.tile([128, 4], F32, tag="t2")
                    t3 = smp.tile([128, 4], F32, tag="t3")
                    y1 = smp.tile([128, 4], F32, tag="y1")
                    nc.vector.tensor_tensor(out=t1, in0=y0, in1=y0, op=ALU.mult)
                    nc.vector.tensor_tensor(out=t2, in0=t1, in1=ssc, op=ALU.mult)
                    nc.vector.tensor_scalar(out=t3, in0=t2, scalar1=-0.5,
                                            scalar2=1.5, op0=ALU.mult, op1=ALU.add)
                    nc.vector.tensor_tensor(out=y1, in0=t3, in1=y0, op=ALU.mult)
                    inv = smp.tile([128, 4], F32, tag="inv")
                    nc.vector.tensor_tensor(out=t1, in0=y1, in1=y1, op=ALU.mult)
                    nc.vector.tensor_tensor(out=t2, in0=t1, in1=ssc, op=ALU.mult)
                    nc.vector.tensor_scalar(out=t3, in0=t2, scalar1=-0.5,
                                            scalar2=1.5, op0=ALU.mult, op1=ALU.add)
                    nc.vector.tensor_tensor(out=inv, in0=t3, in1=y1, op=ALU.mult)

                    if DBG_LVL < 7:
                        continue
                    # h' = hn * inv (layout B), then transpose back
                    hB_new = stp.tile([128, COLS], F32, tag="hB")
                    for j in range(NJ):
                        nc.vector.tensor_scalar(
                            out=hB_new[:, 128 * j:128 * (j + 1)],
                            in0=hnB[:, 128 * j:128 * (j + 1)],
                            scalar1=inv[:, j:j + 1], scalar2=None, op0=ALU.mult)
                    pH = psH.tile([128, COLS], F32, tag="pH")
                    for j in range(NJ):
                        nc.tensor.transpose(out=pH[:, 128 * j:128 * (j + 1)],
                                            in_=hB_new[:, 128 * j:128 * (j + 1)],
                                            identity=ident)
                    hT_new = stp.tile([128, COLS], F32, tag="hT")
                    nc.scalar.copy(out=hT_new, in_=pH)
                    hB, hT = hB_new, hT_new

                # -------- output: h[b,k,:] = hB[(b&1)*64+k, 128*(b>>1)+:] --
                for b in range(BL):
                    src = hB[(b & 1) * 64:(b & 1) * 64 + 64,
                             128 * (b >> 1):128 * (b >> 1) + 128]
                    nc.sync.dma_start(out=out_t[b, :, :], in_=src)
    nc.compile()
    return nc


def _prep_core(core, prgrph, prgrph_mask, embedding_matrix, positional_mask,
               Uw, Vw, Ww, keys):
    b0 = core * BL
    pr = prgrph[b0:b0 + BL]          # [8, T, L]
    pm = prgrph_mask[b0:b0 + BL]
    ky = keys[b0:b0 + BL]            # [8, K, D]

    idx_core = np.ascontiguousarray(pr.transpose(1, 0, 2)).reshape(-1)  # (t,b,l)
    gidx = np.ascontiguousarray(
        idx_core.reshape(NGI, G, 128).transpose(0, 2, 1)).astype(np.int32)

    maskf = pm.transpose(1, 0, 2).reshape(-1).astype(np.float32)
    mw = maskf.reshape(CHUNKS, 4, 32)
    mo = np.zeros((CHUNKS, 128, 4), dtype=np.float32)
    for jj in range(4):
        mo[:, jj * 32:(jj + 1) * 32, jj] = mw[:, jj, :]
    mo = np.ascontiguousarray(
        mo.reshape(NGI, G, 128, 4).transpose(0, 2, 1, 3))

    posrep = np.ascontiguousarray(np.tile(positional_mask, (4, 1))).astype(np.float32)
    keysT = np.ascontiguousarray(ky.transpose(2, 0, 1).reshape(D, COLS))

    # layout-B: partition p, chunk j -> b = 2j + (p>>6)
    p_ar = np.arange(128)
    j_ar = np.arange(4)
    b_of = 2 * j_ar[None, :] + (p_ar[:, None] >> 6)          # [128, 4]
    msent = pm.any(axis=2).astype(np.float32)                # [8, T]
    mscal = np.ascontiguousarray(
        msent[b_of].transpose(0, 2, 1).reshape(128, 4 * T))  # [p, 4t+j]
    oh32 = np.zeros((128, 32), dtype=np.float32)
    for jj in range(4):
        oh32[p_ar, 8 * jj + b_of[:, jj]] = 1.0
    ident = np.eye(128, dtype=np.float32)

    return {
        "emb": np.ascontiguousarray(embedding_matrix.astype(np.float32)),
        "gidx": gidx, "maskones": mo, "posrep": posrep,
        "keysT": keysT,
        "Uw": np.ascontiguousarray(Uw.astype(np.float32)),
        "Vw": np.ascontiguousarray(Vw.astype(np.float32)),
        "Ww": np.ascontiguousarray(Ww.astype(np.float32)),
        "maskscal": mscal, "onehot32": oh32, "ident": ident,
    }


def kernel(prgrph, prgrph_mask, embedding_matrix, positional_mask,
           Uw, Vw, Ww, keys, _trace=False):
    prgrph = np.asarray(prgrph)
    prgrph_mask = np.asarray(prgrph_mask)
    embedding_matrix = np.asarray(embedding_matrix, dtype=np.float32)
    positional_mask = np.asarray(positional_mask, dtype=np.float32)
    Uw = np.asarray(Uw, dtype=np.float32)
    Vw = np.asarray(Vw, dtype=np.float32)
    Ww = np.asarray(Ww, dtype=np.float32)
    keys = np.asarray(keys, dtype=np.float32)

    if "nc" not in _cache:
        _cache["nc"] = _build_nc()
    nc = _cache["nc"]

    in_maps = [_prep_core(c, prgrph, prgrph_mask, embedding_matrix,
                          positional_mask, Uw, Vw, Ww, keys)
               for c in range(NCORES)]
    res = run_bass_kernel_spmd(nc, in_maps, core_ids=list(range(NCORES)),
                               trace=_trace)
    outs = [np.asarray(r["h_out"]).reshape(BL, K, D) for r in res.results]
    full = np.concatenate(outs, axis=0)
    if _trace:
        kernel.last_results = res
    return full



# revision 3
# speedup vs baseline: 65.4628x; 65.4628x over previous
"""Trainium2 Bass kernel v2 for BasicRecurrentEntityEncoder.

Math (per batch b, entity k, step t):
  enc[b,t,:]  = sum_l mask[b,t,l] * emb[prgrph[b,t,l]] * posmask[l,:]
  g           = sigmoid(h.s + keys.s + (active ? 0 : -100))   (mask as bias)
  h_tilda     = sigmoid(h@U + keys@V + s@W)
  h           = normalize(h + g*h_tilda)

Sharding: data-parallel over batch, 8 paragraphs per core.

v2 changes vs baseline:
 - one batched indirect gather per 1024 words (32 instrs instead of 256)
 - scan state and matmuls in bf16 (PE 1 cyc/row + FWL; DVE 2x/4x modes)
 - pre-activation computed directly in layout B (chunked matmuls with
   hT/keysT/onehot as stationary): no h_tilda transpose, ACT sigmoid
   lands straight in layout B
 - s@W term broadcast via onehot matmul (oh8)
 - sentence mask folded into the ks table as a -100 gate bias
 - ss = ||hn||^2 via ACT Square+accum_out (frees DVE)
 - normalize folded into the transpose-back: pH = hnB^T @ diag(inv)
 - 1-Newton rsqrt (magic seed) by default

Layouts:
  layout T: [d=128, c=512], c = b*64+k  (bf16 state hTb)
  layout B: chunk j = c>>7, partition p = c&127; b = 2j + (p>>6), k = p&63
"""
import numpy as np

import concourse.bass as bass
import concourse.bacc as bacc
import concourse.tile as tile
from concourse import mybir
from concourse.bass_utils import run_bass_kernel_spmd

F32 = mybir.dt.float32
BF16 = mybir.dt.bfloat16
I32 = mybir.dt.int32
AF = mybir.ActivationFunctionType
ALU = mybir.AluOpType

B, T, L, D, K, V = 64, 128, 32, 128, 64, 50000
NCORES = 8
BL = B // NCORES              # 8 paragraphs per core
COLS = BL * K                 # 512 state columns per core
NJ = COLS // 128              # 4 layout-B chunks
WORDS = BL * T * L            # 32768 gathered words per core
CHUNKS = WORDS // 128         # 256
G = 8                         # chunks per gather instruction
NGI = CHUNKS // G             # 32 gather instructions
MAGIC = 0x5F3759DF

_cache = {}

DBG_SCAN_T = T
NR_ITERS = 1                  # Newton iterations for rsqrt
GATHER_PAIRS = False          # dma_gather of 512B pairs (8 instrs) vs 256
                              # per-chunk indirect DMAs; the InstDMAGatherAnt
                              # path fails to execute on this runtime build,
                              # so keep per-chunk indirect DMAs
NGI2 = 8                      # dma_gather instruction count
PER = WORDS // NGI2           # words per dma_gather (4096)
DBG_ENC_OUT = False           # extra DRAM output with encT after phase 1


I16 = mybir.dt.int16


def _build_nc(target_bir_lowering=False):
    nc = bacc.Bacc(None, target_bir_lowering=target_bir_lowering,
                   num_swdge_queues=1)

    if GATHER_PAIRS:
        emb_t = nc.dram_tensor("emb", [V // 2, 2 * D], BF16, kind="ExternalInput")
        gidx_t = nc.dram_tensor("gidx", [128, WORDS // 16], I16,
                                kind="ExternalInput")
        moo_t = nc.dram_tensor("maskodd", [128, CHUNKS * 4], BF16,
                               kind="ExternalInput")
        posrep_t = nc.dram_tensor("posrep", [128, 2 * D], BF16,
                                  kind="ExternalInput")
    else:
        emb_t = nc.dram_tensor("emb", [V, D], BF16, kind="ExternalInput")
        gidx_t = nc.dram_tensor("gidx", [128, CHUNKS], I32, kind="ExternalInput")
        posrep_t = nc.dram_tensor("posrep", [128, 128], BF16, kind="ExternalInput")
    mo_t = nc.dram_tensor("maskones", [128, CHUNKS * 4], BF16, kind="ExternalInput")
    keysT_t = nc.dram_tensor("keysT", [128, COLS], F32, kind="ExternalInput")
    keysTb_t = nc.dram_tensor("keysTb", [128, COLS], BF16, kind="ExternalInput")
    U_t = nc.dram_tensor("Uw", [D, D], BF16, kind="ExternalInput")
    V_t = nc.dram_tensor("Vw", [D, D], BF16, kind="ExternalInput")
    W_t = nc.dram_tensor("Ww", [D, D], BF16, kind="ExternalInput")
    ksmb_t = nc.dram_tensor("ksmb", [128, 4 * T], F32, kind="ExternalInput")
    oh_t = nc.dram_tensor("onehot32", [128, 32], F32, kind="ExternalInput")
    oh8_t = nc.dram_tensor("oh8", [8, COLS], BF16, kind="ExternalInput")
    id_t = nc.dram_tensor("identb", [128, 128], BF16, kind="ExternalInput")
    out_t = nc.dram_tensor("h_out", [BL, K, D], F32, kind="ExternalOutput")
    enc_t = (nc.dram_tensor("enc_out", [128, T * BL], F32, kind="ExternalOutput")
             if DBG_ENC_OUT else None)

    with tile.TileContext(nc) as tc:
        with tc.tile_pool(name="persist", bufs=1) as pp:
            posrep = pp.tile([128, 2 * D] if GATHER_PAIRS else [128, D], BF16)
            keysT = pp.tile([128, COLS], F32)
            keysTb = pp.tile([128, COLS], BF16)
            Uwb = pp.tile([D, D], BF16)
            Vwb = pp.tile([D, D], BF16)
            Wwb = pp.tile([D, D], BF16)
            ksmb = pp.tile([128, 4 * T], F32)
            oh32 = pp.tile([128, 32], F32)
            oh8b = pp.tile([8, COLS], BF16)
            identb = pp.tile([128, 128], BF16)
            gidxs = pp.tile([128, WORDS // 16] if GATHER_PAIRS
                            else [128, CHUNKS], I16 if GATHER_PAIRS else I32)
            mos = pp.tile([128, CHUNKS * 4], BF16)
            moso = (pp.tile([128, CHUNKS * 4], BF16, name="moso", tag="moso")
                    if GATHER_PAIRS else None)
            encT = pp.tile([128, T * BL], F32)      # [d, t*8+b]
            encTb = pp.tile([128, T * BL], BF16)
            ksst = pp.tile([128, 4 * T], F32)       # [p, 4t+j]  (incl. -100 bias)
            nc.sync.dma_start(out=posrep, in_=posrep_t[:, :])
            nc.sync.dma_start(out=keysT, in_=keysT_t[:, :])
            nc.sync.dma_start(out=keysTb, in_=keysTb_t[:, :])
            nc.sync.dma_start(out=Uwb, in_=U_t[:, :])
            nc.sync.dma_start(out=Vwb, in_=V_t[:, :])
            nc.sync.dma_start(out=Wwb, in_=W_t[:, :])
            nc.sync.dma_start(out=ksmb, in_=ksmb_t[:, :])
            nc.sync.dma_start(out=oh32, in_=oh_t[:, :])
            nc.sync.dma_start(out=oh8b, in_=oh8_t[:, :])
            nc.sync.dma_start(out=identb, in_=id_t[:, :])
            nc.sync.dma_start(out=gidxs, in_=gidx_t[:, :])
            nc.sync.dma_start(out=mos, in_=mo_t[:, :])
            if GATHER_PAIRS:
                nc.sync.dma_start(out=moso, in_=moo_t[:, :])

            # ---------------- Phase 1: gather + sentence encoder ----------
            if GATHER_PAIRS:
                # gather 512B vocab-row PAIRS (idx>>1 fits int16), select the
                # word's half via split even/odd mask-onehot matmuls
                with tc.tile_pool(name="p1sb", bufs=3) as p1, \
                     tc.tile_pool(name="p1ps", bufs=2, space="PSUM") as p1ps:
                    NCH = PER // 128          # chunks per gather (32)
                    for n in range(NGI2):
                        embp = p1.tile([128, NCH, 2 * D], BF16, tag="embp")
                        nc.gpsimd.dma_gather(
                            out_ap=embp[:, :, :], in_ap=emb_t[:, :],
                            idxs_ap=gidxs[:, n * (PER // 16):
                                          (n + 1) * (PER // 16)],
                            num_idxs=PER, num_idxs_reg=PER,
                            elem_size=2 * D, queue_num=0)
                        wt = p1.tile([128, NCH, 2 * D], BF16, tag="wt")
                        pr_bc = bass.AP(tensor=posrep.tensor,
                                        offset=posrep.offset,
                                        ap=[posrep.ap[0], [0, NCH], [1, 2 * D]])
                        nc.vector.tensor_tensor(out=wt, in0=embp, in1=pr_bc,
                                                op=ALU.mult)
                        penc = p1ps.tile([128, 128], F32, tag="penc")
                        for c in range(NCH):
                            ch = n * NCH + c
                            o4 = (ch % 32) * 4
                            nc.tensor.matmul(
                                out=penc[:, o4:o4 + 4],
                                lhsT=wt[:, c, 0:D],
                                rhs=mos[:, ch * 4:ch * 4 + 4],
                                start=True, stop=False)
                            nc.tensor.matmul(
                                out=penc[:, o4:o4 + 4],
                                lhsT=wt[:, c, D:2 * D],
                                rhs=moso[:, ch * 4:ch * 4 + 4],
                                start=False, stop=True)
                        c0 = n * 128
                        nc.scalar.copy(out=encT[:, c0:c0 + 128], in_=penc)
                        nc.vector.tensor_copy(out=encTb[:, c0:c0 + 128],
                                              in_=penc)
            else:
                with tc.tile_pool(name="p1sb", bufs=3) as p1, \
                     tc.tile_pool(name="p1ps", bufs=2, space="PSUM") as p1ps:
                    penc = None
                    for n in range(NGI):
                        embg = p1.tile([128, G, 128], BF16, tag="embg")
                        for g in range(G):
                            nc.gpsimd.indirect_dma_start(
                                out=embg[:, g, :], out_offset=None,
                                in_=emb_t[:, :],
                                in_offset=bass.IndirectOffsetOnAxis(
                                    ap=gidxs[:, n * G + g:n * G + g + 1],
                                    axis=0))
                        wt = p1.tile([128, G, 128], BF16, tag="wt")
                        pr_bc = bass.AP(tensor=posrep.tensor,
                                        offset=posrep.offset,
                                        ap=[posrep.ap[0], [0, G], [1, 128]])
                        nc.vector.tensor_tensor(out=wt, in0=embg, in1=pr_bc,
                                                op=ALU.mult)
                        for g in range(G):
                            ch = n * G + g
                            if ch % 32 == 0:
                                penc = p1ps.tile([128, 128], F32, tag="penc")
                            nc.tensor.matmul(
                                out=penc[:, (ch % 32) * 4:(ch % 32) * 4 + 4],
                                lhsT=wt[:, g, :],
                                rhs=mos[:, ch * 4:ch * 4 + 4],
                                start=True, stop=True)
                            if ch % 32 == 31:
                                c0 = (ch // 32) * 128
                                nc.scalar.copy(out=encT[:, c0:c0 + 128],
                                               in_=penc)
                                nc.vector.tensor_copy(
                                    out=encTb[:, c0:c0 + 128], in_=penc)

            if DBG_ENC_OUT:
                nc.sync.dma_start(out=enc_t[:, :], in_=encT)

            # ---------------- Phase 1.5: ks table (f32) -------------------
            with tc.tile_pool(name="ksps", bufs=2, space="PSUM") as ksps:
                for b in range(BL):
                    psk = ksps.tile([64, 128], F32, tag="psk")
                    encb = bass.AP(tensor=encT.tensor, offset=encT.offset + b,
                                   ap=[encT.ap[0], [BL, T]])
                    nc.tensor.matmul(out=psk, lhsT=keysT[:, b * 64:(b + 1) * 64],
                                     rhs=encb, start=True, stop=True)
                    r0 = (b & 1) * 64
                    nc.vector.tensor_tensor(
                        out=ksst[r0:r0 + 64, (b >> 1)::4],
                        in0=psk, in1=ksmb[r0:r0 + 64, (b >> 1)::4],
                        op=ALU.add)

            # ---------------- Phase 2: the scan ---------------------------
            with tc.tile_pool(name="st", bufs=2) as stp, \
                 tc.tile_pool(name="sm", bufs=3) as smp, \
                 tc.tile_pool(name="scr", bufs=2) as scrp, \
                 tc.tile_pool(name="psB", bufs=2, space="PSUM") as psB, \
                 tc.tile_pool(name="psG", bufs=2, space="PSUM") as psG, \
                 tc.tile_pool(name="psW", bufs=2, space="PSUM") as psWp, \
                 tc.tile_pool(name="psH", bufs=2, space="PSUM") as psH:
                hTb = stp.tile([128, COLS], BF16, tag="hTb")
                hB = stp.tile([128, COLS], BF16, tag="hB")
                nc.vector.memset(hTb, 0.0)
                nc.vector.memset(hB, 0.0)

                for t in range(DBG_SCAN_T):
                    s_slb = encTb[:, 8 * t:8 * t + 8]
                    # s@W  -> sWb [8, 128] bf16
                    psW = psWp.tile([8, 128], F32, tag="psW")
                    nc.tensor.matmul(out=psW, lhsT=s_slb, rhs=Wwb,
                                     start=True, stop=True)
                    sWb = smp.tile([8, 128], BF16, tag="sWb")
                    nc.scalar.copy(out=sWb, in_=psW)

                    # gate dots: pG2[p, j] = h_c . s_b(c)  via 8 tiny matmuls
                    pG = psG.tile([128, 4], F32, tag="pG")
                    for j in range(NJ):
                        for half in range(2):
                            c0 = 128 * j + 64 * half
                            nc.tensor.matmul(
                                out=pG[64 * half:64 * half + 64, j:j + 1],
                                lhsT=hTb[:, c0:c0 + 64],
                                rhs=encTb[:, 8 * t + 2 * j + half:
                                          8 * t + 2 * j + half + 1],
                                start=True, stop=True)

                    # pre-activation (layout B), per chunk
                    pB = psB.tile([128, COLS], F32, tag="pB")
                    htB = scrp.tile([128, COLS], BF16, tag="htB")
                    for j in range(NJ):
                        sl = slice(128 * j, 128 * (j + 1))
                        nc.tensor.matmul(out=pB[:, sl], lhsT=hTb[:, sl],
                                         rhs=Uwb, start=True, stop=False)
                        nc.tensor.matmul(out=pB[:, sl], lhsT=keysTb[:, sl],
                                         rhs=Vwb, start=False, stop=False)
                        nc.tensor.matmul(out=pB[:, sl], lhsT=oh8b[:, sl],
                                         rhs=sWb, start=False, stop=True)
                    nc.scalar.activation(out=htB, in_=pB, func=AF.Sigmoid)

                    # gate: add ks (+mask bias), sigmoid
                    gks = smp.tile([128, 4], F32, tag="gks")
                    nc.vector.tensor_tensor(out=gks, in0=pG,
                                            in1=ksst[:, 4 * t:4 * t + 4],
                                            op=ALU.add)
                    gm = smp.tile([128, 4], F32, tag="gm")
                    nc.scalar.activation(out=gm, in_=gks, func=AF.Sigmoid)

                    # hn = h + g*h_tilda  (layout B, bf16)
                    hnB = scrp.tile([128, COLS], BF16, tag="hnB")
                    for j in range(NJ):
                        sl = slice(128 * j, 128 * (j + 1))
                        nc.vector.scalar_tensor_tensor(
                            out=hnB[:, sl], in0=htB[:, sl],
                            scalar=gm[:, j:j + 1], in1=hB[:, sl],
                            op0=ALU.mult, op1=ALU.add)

                    # ss = sum_d hn^2: chunks 0-1 on ACT (Square+accum),
                    # chunks 2-3 on DVE (sq + reduce) — balances both engines
                    ss = smp.tile([128, 4], F32, tag="ss")
                    sqs = scrp.tile([128, COLS], BF16, tag="sqs")
                    for j in range(2):
                        sl = slice(128 * j, 128 * (j + 1))
                        nc.scalar.activation(out=sqs[:, sl], in_=hnB[:, sl],
                                             func=AF.Square,
                                             accum_out=ss[:, j:j + 1])
                    nc.vector.tensor_tensor(out=sqs[:, 256:], in0=hnB[:, 256:],
                                            in1=hnB[:, 256:], op=ALU.mult)
                    nc.vector.tensor_reduce(
                        out=ss[:, 2:4],
                        in_=sqs[:, 256:].rearrange("p (a b) -> p a b", b=128),
                        axis=mybir.AxisListType.X, op=ALU.add)

                    # inv = rsqrt(max(ss, 1e-12)): magic seed + NR_ITERS Newton
                    ssc = smp.tile([128, 4], F32, tag="ssc")
                    nc.vector.tensor_scalar(out=ssc, in0=ss, scalar1=1e-12,
                                            scalar2=None, op0=ALU.max)
                    seed = smp.tile([128, 4], I32, tag="seed")
                    nc.vector.tensor_scalar(out=seed, in0=ssc.bitcast(I32),
                                            scalar1=-0.5, scalar2=float(MAGIC),
                                            op0=ALU.mult, op1=ALU.add)
                    y = seed.bitcast(F32)
                    t1 = smp.tile([128, 4], F32, tag="t1")
                    t2 = smp.tile([128, 4], F32, tag="t2")
                    t3 = smp.tile([128, 4], F32, tag="t3")
                    for it in range(NR_ITERS):
                        yn = smp.tile([128, 4], F32, tag=f"y{it}")
                        nc.vector.tensor_tensor(out=t1, in0=y, in1=y, op=ALU.mult)
                        nc.vector.tensor_tensor(out=t2, in0=t1, in1=ssc, op=ALU.mult)
                        nc.vector.tensor_scalar(out=t3, in0=t2, scalar1=-0.5,
                                                scalar2=1.5, op0=ALU.mult,
                                                op1=ALU.add)
                        nc.vector.tensor_tensor(out=yn, in0=t3, in1=y, op=ALU.mult)
                        y = yn
                    inv = y

                    # normalized h'T via fused transpose: pH_j = hnB_j^T @ diag(inv_j)
                    # Dinv[p, 128j+q] = (p==q) ? inv[p,j] : 0, one gpsimd op
                    Dinv = scrp.tile([128, NJ, 128], BF16, tag="Dinv")
                    inv_bc = bass.AP(tensor=inv.tensor, offset=inv.offset,
                                     ap=[inv.ap[0], [1, NJ], [0, 128]])
                    nc.gpsimd.affine_select(
                        out=Dinv, in_=inv_bc,
                        pattern=[[0, NJ], [-1, 128]],
                        compare_op=ALU.is_equal, fill=0.0,
                        base=0, channel_multiplier=1)
                    pH = psH.tile([128, COLS], F32, tag="pH")
                    hTb_new = stp.tile([128, COLS], BF16, tag="hTb")
                    for j in range(NJ):
                        sl = slice(128 * j, 128 * (j + 1))
                        nc.tensor.matmul(out=pH[:, sl], lhsT=hnB[:, sl],
                                         rhs=Dinv[:, j, :], start=True, stop=True)
                    for h2 in range(2):
                        sl = slice(256 * h2, 256 * (h2 + 1))
                        nc.scalar.copy(out=hTb_new[:, sl], in_=pH[:, sl])

                    # normalized h'B (off critical path)
                    hB_new = stp.tile([128, COLS], BF16, tag="hB")
                    for j in range(NJ):
                        sl = slice(128 * j, 128 * (j + 1))
                        nc.vector.tensor_scalar(out=hB_new[:, sl],
                                                in0=hnB[:, sl],
                                                scalar1=inv[:, j:j + 1],
                                                scalar2=None, op0=ALU.mult)
                    hB, hTb = hB_new, hTb_new

                # -------- output: h[b,k,:] = hB[(b&1)*64+k, 128*(b>>1)+:] --
                houtf = pp.tile([128, COLS], F32)
                nc.vector.tensor_copy(out=houtf, in_=hB)
                for b in range(BL):
                    src = houtf[(b & 1) * 64:(b & 1) * 64 + 64,
                                128 * (b >> 1):128 * (b >> 1) + 128]
                    nc.sync.dma_start(out=out_t[b, :, :], in_=src)
    nc.compile()
    return nc


def _prep_core(core, prgrph, prgrph_mask, embedding_matrix, positional_mask,
               Uw, Vw, Ww, keys):
    b0 = core * BL
    pr = prgrph[b0:b0 + BL]          # [8, T, L]
    pm = prgrph_mask[b0:b0 + BL]
    ky = keys[b0:b0 + BL]            # [8, K, D]

    idx_core = np.ascontiguousarray(pr.transpose(1, 0, 2)).reshape(-1)  # (t,b,l)

    def build_mos(maskf):
        mw = maskf.reshape(CHUNKS, 4, 32)
        mo = np.zeros((CHUNKS, 128, 4), dtype=np.float32)
        for jj in range(4):
            mo[:, jj * 32:(jj + 1) * 32, jj] = mw[:, jj, :]
        # -> [128, CHUNKS*4] with col = 4*ch + jj
        return np.ascontiguousarray(mo.transpose(1, 0, 2).reshape(128, CHUNKS * 4))

    maskf = pm.transpose(1, 0, 2).reshape(-1).astype(np.float32)
    if GATHER_PAIRS:
        idxp = (idx_core >> 1).astype(np.int16)
        gidx = np.ascontiguousarray(np.tile(idxp.reshape(-1, 16).T, (8, 1)))
        par = (idx_core & 1).astype(np.float32)
        mos = build_mos(maskf * (1.0 - par))
        moso = build_mos(maskf * par)
        posrep = np.ascontiguousarray(
            np.tile(positional_mask, (4, 2))).astype(np.float32)
    else:
        gidx = np.ascontiguousarray(
            idx_core.reshape(CHUNKS, 128).T).astype(np.int32)  # [128, CHUNKS]
        mos = build_mos(maskf)
        moso = None
        posrep = np.ascontiguousarray(
            np.tile(positional_mask, (4, 1))).astype(np.float32)
    keysT = np.ascontiguousarray(ky.transpose(2, 0, 1).reshape(D, COLS))

    # layout-B: partition p, chunk j -> b = 2j + (p>>6)
    p_ar = np.arange(128)
    j_ar = np.arange(4)
    b_of = 2 * j_ar[None, :] + (p_ar[:, None] >> 6)          # [128, 4]
    msent = pm.any(axis=2)                                   # [8, T]
    ksmb = np.ascontiguousarray(
        np.where(msent[b_of], 0.0, -100.0).transpose(0, 2, 1)
        .reshape(128, 4 * T)).astype(np.float32)             # [p, 4t+j]
    oh32 = np.zeros((128, 32), dtype=np.float32)
    for jj in range(4):
        oh32[p_ar, 8 * jj + b_of[:, jj]] = 1.0
    c_ar = np.arange(COLS)
    b_of_c = 2 * (c_ar >> 7) + ((c_ar & 127) >> 6)           # [512]
    oh8 = (np.arange(8)[:, None] == b_of_c[None, :])

    def bf(x):
        import jax.numpy as jnp
        return np.asarray(jnp.asarray(x, dtype=jnp.bfloat16))

    emb_host = bf(embedding_matrix)
    if GATHER_PAIRS:
        emb_host = np.ascontiguousarray(emb_host.reshape(V // 2, 2 * D))
    d = {
        "emb": emb_host,
        "gidx": gidx, "maskones": bf(mos), "posrep": bf(posrep),
        "keysT": keysT, "keysTb": bf(keysT),
        "Uw": bf(Uw), "Vw": bf(Vw), "Ww": bf(Ww),
        "ksmb": ksmb, "onehot32": oh32, "oh8": bf(oh8.astype(np.float32)),
        "identb": bf(np.eye(128, dtype=np.float32)),
    }
    if GATHER_PAIRS:
        d["maskodd"] = bf(moso)
    return d


def kernel(prgrph, prgrph_mask, embedding_matrix, positional_mask,
           Uw, Vw, Ww, keys, _trace=False):
    prgrph = np.asarray(prgrph)
    prgrph_mask = np.asarray(prgrph_mask)
    embedding_matrix = np.asarray(embedding_matrix, dtype=np.float32)
    positional_mask = np.asarray(positional_mask, dtype=np.float32)
    Uw = np.asarray(Uw, dtype=np.float32)
    Vw = np.asarray(Vw, dtype=np.float32)
    Ww = np.asarray(Ww, dtype=np.float32)
    keys = np.asarray(keys, dtype=np.float32)

    if "nc" not in _cache:
        _cache["nc"] = _build_nc()
    nc = _cache["nc"]

    in_maps = [_prep_core(c, prgrph, prgrph_mask, embedding_matrix,
                          positional_mask, Uw, Vw, Ww, keys)
               for c in range(NCORES)]
    res = run_bass_kernel_spmd(nc, in_maps, core_ids=list(range(NCORES)),
                               trace=_trace)
    outs = [np.asarray(r["h_out"]).reshape(BL, K, D) for r in res.results]
    full = np.concatenate(outs, axis=0)
    if _trace:
        kernel.last_results = res
    return full
